# revision 1
# baseline (speedup 1.0000x reference)
"""FluxSingleTransformerBlock on 8 trn2 NeuronCores.

Sharding: tensor-parallel over heads (3/core) and mlp_hidden (1536/core);
norm_w row-sharded with a tiny AllGather of emb; out-proj row-parallel with
chunked bf16 ReduceScatter overlapped with compute; final gate+residual on
each core's sequence shard; host concatenates disjoint shards.

Layout: activations are [feature_partitions, seq_free]; host pre-transposes
x and all weights.  Matmuls run in bf16 (1 cyc/row on the PE); statistics in
fp32.  LN mean/var and softmax row sums are partition-dim reductions done as
all-ones matmuls on the PE.  Rope pairs are host-de-interleaved into
(evens|odds) rows of the q/k projection output.
"""

import os
import sys

for _p in ("/opt/trn_rl_repo", "/root/.axon_site/_ro/trn_rl_repo"):
    if os.path.isdir(_p) and _p not in sys.path:
        sys.path.append(_p)

import numpy as np

import concourse.bass as bass
import concourse.bacc as bacc
import concourse.mybir as mybir
import concourse.tile as tile
import concourse.masks as masks
from concourse.bass_utils import run_bass_kernel_spmd

FP32 = mybir.dt.float32
BF16 = mybir.dt.bfloat16
AF = mybir.ActivationFunctionType
ALU = mybir.AluOpType

B, S, D, H, DH = 1, 2048, 3072, 24, 128
M = 4 * D  # 12288
NC = 8
HPC = H // NC            # heads per core = 3
DQ = HPC * DH            # q/k/v cols per core = 384
MPC = M // NC            # mlp rows per core = 1536
MT = MPC // 128          # 12
NPC = 3 * D // NC        # norm rows per core = 1152
NJ = NPC // 128          # 9
FPC = DQ + MPC           # fused contraction rows per core = 1920
FT = FPC // 128          # 15
KT = D // 128            # 24 d_in tiles
ST = S // 128            # 16 seq tiles
NCH = 4                  # seq chunks
RSC = 8                  # reduce-scatter chunks
RW = S // RSC            # 256 rows per rs chunk
RWC = RW // NC           # 32 rows per core per rs chunk
CW = S // NCH            # 512
SSH = S // NC            # 256 final rows per core
EPS = 1e-6
ISQD = float(1.0 / np.sqrt(DH))

_CACHE = {}


def _build(debug=False):
    key = ("nc", debug)
    if key in _CACHE:
        return _CACHE[key]
    nc = bacc.Bacc("TRN2", target_bir_lowering=False, debug=False, num_devices=NC)

    def din(name, shape, dt=BF16):
        return nc.dram_tensor(name, list(shape), dt, kind="ExternalInput").ap()

    def dodbg(name, shape, dt=BF16):
        return nc.dram_tensor(name, list(shape), dt, kind="ExternalOutput").ap()

    io = {
        "xTb": din("xTb", [D, S]),
        "xres": din("xres", [SSH, D], FP32),
        "tembT": din("tembT", [128, KT], FP32),
        "outb_row": din("outb_row", [1, D], FP32),
        "normT": din("normT", [D, NPC]),
        "normb": din("normb", [128, NJ], FP32),
        "qwT": din("qwT", [D, DQ]),
        "kwT": din("kwT", [D, DQ]),
        "vwT": din("vwT", [D, DQ]),
        "qb": din("qb", [128, HPC], FP32),
        "kb": din("kb", [128, HPC], FP32),
        "vb": din("vb", [128, HPC], FP32),
        "qrw": din("qrw", [128, 1], FP32),
        "krw": din("krw", [128, 1], FP32),
        "mlpT": din("mlpT", [MT, 128, D]),
        "mlpb": din("mlpb", [128, MT], FP32),
        "outT": din("outT", [FPC, D]),
        "cosP": din("cosP", [128, S]),
        "sinP": din("sinP", [128, S]),
        "y": nc.dram_tensor("y", [SSH, D], FP32, kind="ExternalOutput").ap(),
    }
    io["dbg"] = {}
    if debug:
        io["dbg"] = {
            "d_emb": dodbg("d_emb", [128, NJ], FP32),
            "d_shift": dodbg("d_shift", [128, KT], FP32),
            "d_gate": dodbg("d_gate", [128, KT], FP32),
            "d_mu": dodbg("d_mu", [128, S], FP32),
            "d_rstd": dodbg("d_rstd", [128, S], FP32),
            "d_nx0": dodbg("d_nx0", [128, S]),
            "d_q0": dodbg("d_q0", [128, S]),
            "d_k0": dodbg("d_k0", [128, S]),
            "d_v0": dodbg("d_v0", [128, S]),
            "d_p0": dodbg("d_p0", [128, CW]),
            "d_o0": dodbg("d_o0", [128, S]),
            "d_g0": dodbg("d_g0", [128, S]),
            "d_op0": dodbg("d_op0", [128, CW]),
        }

    with tile.TileContext(nc) as tc:
        _emit(tc, io)
    nc.compile()
    _CACHE[key] = nc
    return nc


def _emit(tc, io):
    from contextlib import ExitStack

    with ExitStack() as ctx:
        _emit_body(ctx, tc, io)


def _emit_body(ctx, tc, io):
    nc = tc.nc
    dbg = io["dbg"]
    debug = bool(dbg)

    def dma(out, in_):
        nc.sync.dma_start(out=out, in_=in_)

    # ---------- constants ----------
    const = ctx.enter_context(tc.tile_pool(name="const", bufs=1))
    ones_b = const.tile([128, 128], BF16)
    nc.vector.memset(ones_b[:], 1.0)
    ident_b = const.tile([128, 128], BF16)
    masks.make_identity(nc, ident_b[:])
    ident_f = const.tile([128, 128], FP32)
    masks.make_identity(nc, ident_f[:])
    cos_t = const.tile([128, S], BF16)
    sin_t = const.tile([128, S], BF16)
    dma(cos_t[:], io["cosP"][:, :])
    dma(sin_t[:], io["sinP"][:, :])
    tembT_t = const.tile([128, KT], FP32)
    dma(tembT_t[:], io["tembT"][:, :])
    qrw_t = const.tile([128, 1], FP32)
    krw_t = const.tile([128, 1], FP32)
    dma(qrw_t[:], io["qrw"][:, :])
    dma(krw_t[:], io["krw"][:, :])
    qb_t = const.tile([128, HPC], FP32)
    kb_t = const.tile([128, HPC], FP32)
    vb_t = const.tile([128, HPC], FP32)
    dma(qb_t[:], io["qb"][:, :])
    dma(kb_t[:], io["kb"][:, :])
    dma(vb_t[:], io["vb"][:, :])
    mlpb_t = const.tile([128, MT], FP32)
    dma(mlpb_t[:], io["mlpb"][:, :])
    normb_t = const.tile([128, NJ], FP32)
    dma(normb_t[:], io["normb"][:, :])
    eps_t = const.tile([128, 1], FP32)
    nc.vector.memset(eps_t[:], EPS)

    dram = ctx.enter_context(tc.tile_pool(name="dram", bufs=1, space="DRAM"))
    nx_dram = dram.tile([KT, 128, S], BF16, tag="nxd", name="nx_dram")
    fb_dram = dram.tile([KT, 128, S], BF16, tag="fbd", name="fb_dram")
    qkv_dram = dram.tile([3, HPC, 128, S], BF16, tag="qkvd", name="qkv_dram")
    adaln = ctx.enter_context(tc.tile_pool(name="adaln", bufs=1))
    shift_c = adaln.tile([128, KT], FP32)
    scale1_c = adaln.tile([128, KT], FP32)
    gate_c = adaln.tile([128, KT], FP32)

    # ---------- phase 0: emb = silu(temb) @ norm_wT; AllGather ----------
    with (
        tc.tile_pool(name="p0", bufs=1) as p0,
        tc.tile_pool(name="p0w", bufs=KT) as p0w,
        tc.tile_pool(name="p0p", bufs=1, space="PSUM") as p0p,
    ):
        st_t = p0.tile([128, KT], BF16)
        nc.scalar.activation(st_t[:], tembT_t[:], AF.Silu)
        wslab = []
        for t in range(KT):
            w = p0w.tile([128, NPC], BF16, tag="nslab")
            dma(w[:], io["normT"][bass.ts(t, 128), :])
            wslab.append(w)
        emb_ps = p0p.tile([128, NJ], FP32)
        for j in range(NJ):
            for t in range(KT):
                nc.tensor.matmul(
                    emb_ps[:, j : j + 1],
                    wslab[t][:, bass.ts(j, 128)],
                    st_t[:, t : t + 1],
                    start=(t == 0),
                    stop=(t == KT - 1),
                )
        emb_sb = p0.tile([128, NJ], FP32)
        nc.vector.tensor_add(emb_sb[:], emb_ps[:], normb_t[:])
        if debug:
            dma(dbg["d_emb"][:, :], emb_sb[:])
        emb_loc = dram.tile([NJ, 128], FP32, tag="embloc")
        dma(emb_loc[:, :].rearrange("j p -> p j"), emb_sb[:])
        emb_all = dram.tile([NJ * NC, 128], FP32, tag="emball")
        nc.gpsimd.collective_compute(
            "AllGather",
            ALU.bypass,
            replica_groups=[list(range(NC))],
            ins=[emb_loc.opt()],
            outs=[emb_all.opt()],
        )
        dma(shift_c[:], emb_all[0:KT, :].rearrange("t p -> p t"))
        dma(scale1_c[:], emb_all[KT : 2 * KT, :].rearrange("t p -> p t"))
        dma(gate_c[:], emb_all[2 * KT : 3 * KT, :].rearrange("t p -> p t"))
        nc.vector.tensor_scalar_add(scale1_c[:], scale1_c[:], 1.0)
        if debug:
            dma(dbg["d_shift"][:, :], shift_c[:])
            dma(dbg["d_gate"][:, :], gate_c[:])

    # ---------- phase 1: layernorm stats ----------
    RSTDb = adaln.tile([128, S], BF16, tag="rstdb")
    with (
        tc.tile_pool(name="p1", bufs=1) as p1,
        tc.tile_pool(name="p1x", bufs=2) as p1x,
        tc.tile_pool(name="p1f", bufs=3) as p1f,
        tc.tile_pool(name="p1p", bufs=1, space="PSUM") as p1p,
    ):
        C1 = p1.tile([128, S], FP32, tag="c1")
        for half in range(2):
            hsl = bass.ds(half * (S // 2), S // 2)
            sum_ps = p1p.tile([128, S // 2], FP32, tag="sum")
            ssq_ps = p1p.tile([128, S // 2], FP32, tag="ssq")
            for t in range(KT):
                xt = p1x.tile([128, S // 2], BF16, tag="xin")
                dma(xt[:], io["xTb"][bass.ts(t, 128), hsl])
                sq = p1x.tile([128, S // 2], BF16, tag="xsq")
                nc.scalar.activation(sq[:], xt[:], AF.Square)
                for cc in range(2):
                    csl = bass.ts(cc, CW)
                    nc.tensor.matmul(
                        sum_ps[:, csl], ones_b[:], xt[:, csl],
                        start=(t == 0), stop=(t == KT - 1),
                    )
                    nc.tensor.matmul(
                        ssq_ps[:, csl], ones_b[:], sq[:, csl],
                        start=(t == 0), stop=(t == KT - 1),
                    )
            mu = p1.tile([128, S // 2], FP32, tag="mu")
            nc.scalar.activation(mu[:], sum_ps[:], AF.Copy, scale=1.0 / D)
            musq = p1.tile([128, S // 2], FP32, tag="musq")
            nc.vector.scalar_tensor_tensor(
                musq[:], mu[:], 0.0, mu[:], op0=ALU.bypass, op1=ALU.mult
            )
            nc.vector.tensor_scalar_add(musq[:], musq[:], -EPS)
            vare = p1.tile([128, S // 2], FP32, tag="vare")
            nc.vector.scalar_tensor_tensor(
                vare[:], ssq_ps[:], 1.0 / D, musq[:],
                op0=ALU.mult, op1=ALU.subtract,
            )
            rinv = p1.tile([128, S // 2], FP32, tag="rinv")
            rstd = p1.tile([128, S // 2], FP32, tag="rstd")
            nc.vector.reciprocal_approx_accurate(rinv[:], vare[:], rstd[:])
            nc.scalar.activation(rstd[:], rinv[:], AF.Sqrt)
            nc.vector.tensor_copy(RSTDb[:, hsl], rstd[:])
            nc.vector.tensor_mul(C1[:, hsl], mu[:], rstd[:])
            for t in range(KT):
                fbt = p1f.tile([128, S // 2], BF16, tag="fbt")
                nc.vector.tensor_scalar(
                    fbt[:], C1[:, hsl], scale1_c[:, t : t + 1],
                    shift_c[:, t : t + 1],
                    op0=ALU.mult, op1=ALU.subtract,
                )
                dma(fb_dram[t, :, hsl], fbt[:])
            if debug:
                dma(dbg["d_mu"][:, hsl], mu[:])
                dma(dbg["d_rstd"][:, hsl], rstd[:])
    # ---------- phase 2: LN apply + qkv + rmsnorm + rope (chunk pairs) ----
    W2 = 2 * CW
    with (
        tc.tile_pool(name="p2w", bufs=3 * KT) as p2w,
        tc.tile_pool(name="p2n", bufs=KT + 14) as p2n,
        tc.tile_pool(name="p2x", bufs=2) as p2x,
        tc.tile_pool(name="p2", bufs=2) as p2,
        tc.tile_pool(name="p2f", bufs=1) as p2f,
        tc.tile_pool(name="p2p", bufs=2, space="PSUM") as p2p,
        tc.tile_pool(name="p2p2", bufs=2, space="PSUM") as p2p2,
    ):
        wts = []
        for wap in (io["qwT"], io["kwT"], io["vwT"]):
            wt = []
            for t in range(KT):
                w = p2w.tile([128, DQ], BF16, tag="wqkv")
                dma(w[:], wap[bass.ts(t, 128), :])
                wt.append(w)
            wts.append(wt)
        for cp in range(NCH // 2):
            cpsl = bass.ds(cp * W2, W2)
            # LN apply for the two chunks of this pair
            nxc = []
            for t in range(KT):
                xt = p2x.tile([128, W2], BF16, tag="xin2")
                dma(xt[:], io["xTb"][bass.ts(t, 128), cpsl])
                fb = p2x.tile([128, W2], BF16, tag="fneg")
                dma(fb[:], fb_dram[t, :, cpsl])
                t2 = p2x.tile([128, W2], BF16, tag="t2")
                nc.vector.scalar_tensor_tensor(
                    t2[:], xt[:], scale1_c[:, t : t + 1], RSTDb[:, cpsl],
                    op0=ALU.mult, op1=ALU.mult,
                )
                nx = p2n.tile([128, W2], BF16, tag="nxc")
                nc.vector.tensor_sub(nx[:], t2[:], fb[:])
                dma(nx_dram[t, :, cpsl], nx[:])
                if debug and t == 0:
                    dma(dbg["d_nx0"][:, cpsl], nx[:])
                nxc.append(nx)
            for pi, bt in enumerate((qb_t, kb_t, vb_t)):
                for m in range(HPC):
                    ps = p2p.tile([128, W2], FP32, tag="qkvps")
                    for t in range(KT):
                        for c2 in range(2):
                            nc.tensor.matmul(
                                ps[:, bass.ts(c2, CW)],
                                wts[pi][t][:, bass.ts(m, 128)],
                                nxc[t][:, bass.ts(c2, CW)],
                                start=(t == 0), stop=(t == KT - 1),
                            )
                    if pi == 2:
                        vsb = p2.tile([128, W2], BF16, tag="qsb")
                        nc.scalar.activation(
                            vsb[:], ps[:], AF.Identity, bias=bt[:, m : m + 1]
                        )
                        dma(qkv_dram[pi, m, :, cpsl], vsb[:])
                        continue
                    qsb = p2.tile([128, W2], BF16, tag="qsb")
                    nc.scalar.activation(
                        qsb[:], ps[:], AF.Identity, bias=bt[:, m : m + 1]
                    )
                    sq = p2.tile([128, W2], BF16, tag="sq2")
                    nc.scalar.activation(
                        sq[:], ps[:], AF.Square, bias=bt[:, m : m + 1]
                    )
                    ssq = p2p2.tile([128, W2], FP32, tag="ssq2")
                    for c2 in range(2):
                        nc.tensor.matmul(
                            ssq[:, bass.ts(c2, CW)], ones_b[:],
                            sq[:, bass.ts(c2, CW)], start=True, stop=True,
                        )
                    vare = p2f.tile([128, W2], FP32, tag="vare2")
                    nc.scalar.activation(
                        vare[:], ssq[:], AF.Identity, bias=eps_t[:, 0:1],
                        scale=1.0 / DH,
                    )
                    rinv = p2f.tile([128, W2], FP32, tag="rinv2")
                    rst = p2f.tile([128, W2], FP32, tag="rst2")
                    nc.vector.reciprocal_approx_accurate(rinv[:], vare[:], rst[:])
                    nc.scalar.activation(rst[:], rinv[:], AF.Sqrt)
                    qn = p2.tile([128, W2], BF16, tag="qn")
                    rw = qrw_t if pi == 0 else krw_t
                    nc.vector.scalar_tensor_tensor(
                        qn[:], qsb[:], rw[:, 0:1], rst[:],
                        op0=ALU.mult, op1=ALU.mult,
                    )
                    # rope: out = qn*cos2 + swap(qn)*sin2, sin2 = [-s; s]
                    qsw = p2.tile([128, W2], BF16, tag="qsw")
                    nc.vector.tensor_copy(qsw[0:64, :], qn[64:128, :])
                    nc.vector.tensor_copy(qsw[64:128, :], qn[0:64, :])
                    te = p2.tile([128, W2], BF16, tag="te")
                    to = p2.tile([128, W2], BF16, tag="to")
                    nc.vector.tensor_mul(te[:], qn[:], cos_t[:, cpsl])
                    nc.vector.tensor_mul(to[:], qsw[:], sin_t[:, cpsl])
                    nc.vector.tensor_add(te[:], te[:], to[:])
                    dma(qkv_dram[pi, m, :, cpsl], te[:])
        if debug:
            dma(dbg["d_q0"][:, :], qkv_dram[0, 0, :, :])
            dma(dbg["d_k0"][:, :], qkv_dram[1, 0, :, :])
            dma(dbg["d_v0"][:, :], qkv_dram[2, 0, :, :])

    # ---------- phase 3: mlp -> gT (chunk pairs, shared ldweights) ----
    gp_cm = tc.tile_pool(name="g", bufs=1)
    gp = gp_cm.__enter__()
    gT = [gp.tile([128, S], BF16, tag=f"g{m}", name=f"g{m}") for m in range(MT)]
    with (
        tc.tile_pool(name="p3w", bufs=3) as p3w,
        tc.tile_pool(name="p3n", bufs=KT + 4) as p3n,
        tc.tile_pool(name="p3p", bufs=2, space="PSUM") as p3p,
    ):
        W2 = 2 * CW
        for cp in range(NCH // 2):
            cpsl = bass.ds(cp * W2, W2)
            nxc = []
            for t in range(KT):
                nt = p3n.tile([128, W2], BF16, tag="nxc3")
                dma(nt[:], nx_dram[t, :, cpsl])
                nxc.append(nt)
            for m in range(MT):
                wm = p3w.tile([128, D], BF16, tag="wmlp")
                dma(wm[:], io["mlpT"][m, :, :])
                ps = p3p.tile([128, W2], FP32, tag="mlpps")
                for t in range(KT):
                    for c2 in range(2):
                        nc.tensor.matmul(
                            ps[:, bass.ts(c2, CW)], wm[:, bass.ts(t, 128)],
                            nxc[t][:, bass.ts(c2, CW)],
                            start=(t == 0), stop=(t == KT - 1),
                        )
                nc.scalar.activation(
                    gT[m][:, cpsl], ps[:], AF.Gelu_apprx_tanh,
                    bias=mlpb_t[:, m : m + 1],
                )
        if debug:
            dma(dbg["d_g0"][:, :], gT[0][:])

    # ---------- phase 4: attention -> oT ----------
    qkvp_cm = tc.tile_pool(name="qkv", bufs=1)
    qkvp = qkvp_cm.__enter__()
    qT = [qkvp.tile([128, S], BF16, tag=f"q{m}", name=f"q{m}") for m in range(HPC)]
    kT = [qkvp.tile([128, S], BF16, tag=f"k{m}", name=f"k{m}") for m in range(HPC)]
    vT = [qkvp.tile([128, S], BF16, tag=f"v{m}", name=f"v{m}") for m in range(HPC)]
    for m in range(HPC):
        dma(qT[m][:], qkv_dram[0, m, :, :])
        dma(kT[m][:], qkv_dram[1, m, :, :])
        dma(vT[m][:], qkv_dram[2, m, :, :])
    op_cm = tc.tile_pool(name="o", bufs=1)
    op = op_cm.__enter__()
    oT = [op.tile([128, S], BF16, tag=f"o{m}", name=f"o{m}") for m in range(HPC)]
    with tc.tile_pool(name="vt", bufs=HPC * ST) as vtp:
        vts = {}
        with tc.tile_pool(name="p4vp", bufs=4, space="PSUM") as p4vp:
            for h in range(HPC):
                for kt in range(ST):
                    tp = p4vp.tile([128, 128], BF16, tag="vtp")
                    nc.tensor.transpose(
                        tp[:], vT[h][:, bass.ts(kt, 128)], ident_b[:]
                    )
                    vs = vtp.tile([128, 128], BF16, tag="vts")
                    nc.vector.tensor_copy(vs[:], tp[:])
                    vts[(h, kt)] = vs
        _p4cm = (
            tc.tile_pool(name="p4", bufs=2),
            tc.tile_pool(name="p4pt", bufs=2 * ST),
            tc.tile_pool(name="p4p", bufs=3, space="PSUM"),
            tc.tile_pool(name="p4pa", bufs=2, space="PSUM"),
        )
        p4, p4pt, p4p, p4pa = [cm.__enter__() for cm in _p4cm]
        for h in range(HPC):
            for c in range(NCH):
                csl = bass.ts(c, CW)
                pts = []
                for kt in range(ST):
                    sps = p4p.tile([128, CW], FP32, tag="sps")
                    nc.tensor.matmul(
                        sps[:], kT[h][:, bass.ts(kt, 128)], qT[h][:, csl],
                        start=True, stop=True,
                    )
                    pt = p4pt.tile([128, CW], BF16, tag="pt")
                    nc.scalar.activation(pt[:], sps[:], AF.Exp, scale=ISQD)
                    pts.append(pt)
                if debug and h == 0 and c == 0:
                    dma(dbg["d_p0"][:, :], pts[0][:])
                ops = p4pa.tile([128, CW], FP32, tag="ops")
                sms = p4pa.tile([128, CW], FP32, tag="sms")
                for kt in range(ST):
                    nc.tensor.matmul(
                        ops[:], vts[(h, kt)][:], pts[kt][:],
                        start=(kt == 0), stop=(kt == ST - 1),
                    )
                for kt in range(ST):
                    nc.tensor.matmul(
                        sms[:], ones_b[:], pts[kt][:],
                        start=(kt == 0), stop=(kt == ST - 1),
                    )
                rec = p4.tile([128, CW], FP32, tag="rec")
                scr = p4.tile([128, CW], FP32, tag="scr4")
                nc.vector.reciprocal_approx_accurate(rec[:], sms[:], scr[:])
                nc.vector.tensor_mul(oT[h][:, csl], ops[:], rec[:])
        if debug:
            dma(dbg["d_o0"][:, :], oT[0][:])

        for cm in reversed(_p4cm):
            cm.__exit__(None, None, None)

    # ---------- phase 5: out proj + 8-way chunked ReduceScatter ----------
    fusedT = oT + gT
    rs_out = []
    with (
        tc.tile_pool(name="p5w", bufs=FT) as p5w,
        tc.tile_pool(name="p5", bufs=4) as p5,
        tc.tile_pool(name="p5p", bufs=2, space="PSUM") as p5p,
    ):
        wo = []
        for f in range(FT):
            w = p5w.tile([128, D], BF16, tag="wout")
            dma(w[:], io["outT"][bass.ts(f, 128), :])
            wo.append(w)
        for i in range(RSC):
            rs_in_i = dram.tile([RW, D], BF16, tag=f"rsin{i}", name=f"rsin{i}")
            rs_out_i = dram.tile([RWC, D], BF16, tag=f"rsout{i}",
                                 name=f"rsout{i}")
            rs_out.append(rs_out_i)
            for sblk in range(RW // 128):
                stile = i * (RW // 128) + sblk
                ssl = bass.ts(stile, 128)
                for dcp in range(D // (2 * CW)):
                    ps = p5p.tile([128, 2 * CW], FP32, tag="ops5")
                    for f in range(FT):
                        for c2 in range(2):
                            nc.tensor.matmul(
                                ps[:, bass.ts(c2, CW)], fusedT[f][:, ssl],
                                wo[f][:, bass.ds(dcp * 2 * CW + c2 * CW, CW)],
                                start=(f == 0), stop=(f == FT - 1),
                            )
                    osb = p5.tile([128, 2 * CW], BF16, tag="osb")
                    nc.scalar.activation(osb[:], ps[:], AF.Copy)
                    if debug and stile == 0 and dcp == 0:
                        dma(dbg["d_op0"][:, :], osb[:, 0:CW])
                    dma(rs_in_i[bass.ts(sblk, 128), bass.ts(dcp, 2 * CW)],
                        osb[:])
            nc.gpsimd.collective_compute(
                "ReduceScatter",
                ALU.add,
                replica_groups=[list(range(NC))],
                ins=[rs_in_i.opt()],
                outs=[rs_out_i.opt()],
            )

    op_cm.__exit__(None, None, None)
    qkvp_cm.__exit__(None, None, None)
    gp_cm.__exit__(None, None, None)

    # ---------- phase 6: gate + residual on own shard ----------
    with (
        tc.tile_pool(name="p6", bufs=1) as p6,
        tc.tile_pool(name="p6b", bufs=2) as p6b,
        tc.tile_pool(name="p6p", bufs=2, space="PSUM") as p6p,
    ):
        grow_ps = p6p.tile([KT, 128], FP32, tag="grow")
        nc.tensor.transpose(grow_ps[:], gate_c[:], ident_f[:])
        grow_sb = p6.tile([KT, 128], FP32, tag="growt")
        nc.scalar.activation(grow_sb[:], grow_ps[:], AF.Copy)
        grow = p6.tile([1, D], FP32, tag="growsb")
        dma(grow[0:1, :], grow_sb[:, :])
        ob_sb = p6.tile([1, D], FP32, tag="obrow")
        dma(ob_sb[:], io["outb_row"][:, :])
        hrow = p6.tile([1, D], FP32, tag="hrow")
        nc.vector.tensor_mul(hrow[:], grow[:], ob_sb[:])
        growb = p6.tile([1, D], BF16, tag="growb")
        hrowb = p6.tile([1, D], BF16, tag="hrowb")
        nc.vector.tensor_copy(growb[:], grow[:])
        nc.vector.tensor_copy(hrowb[:], hrow[:])
        GATE = p6.tile([128, D], FP32, tag="gateb")
        HB = p6.tile([128, D], FP32, tag="hb")
        for dc in range(D // CW):
            dsl = bass.ts(dc, CW)
            bps = p6p.tile([128, CW], FP32, tag="bps")
            nc.tensor.matmul(
                bps[:], ones_b[0:1, :], growb[0:1, dsl], start=True, stop=True
            )
            nc.scalar.activation(GATE[:, dsl], bps[:], AF.Copy)
            bps2 = p6p.tile([128, CW], FP32, tag="bps2")
            nc.tensor.matmul(
                bps2[:], ones_b[0:1, :], hrowb[0:1, dsl], start=True, stop=True
            )
            nc.scalar.activation(HB[:, dsl], bps2[:], AF.Copy)
        for half in range(SSH // 128):
            sh = p6b.tile([128, D], BF16, tag="shard")
            for i in range(128 // RWC):
                blk = half * (128 // RWC) + i
                dma(sh[bass.ts(i, RWC), :], rs_out[blk][:, :])
            xr = p6b.tile([128, D], FP32, tag="xr")
            dma(xr[:], io["xres"][bass.ts(half, 128), :])
            t1 = p6b.tile([128, D], FP32, tag="t1")
            nc.vector.tensor_mul(t1[:], sh[:], GATE[:])
            nc.vector.tensor_add(t1[:], t1[:], HB[:])
            nc.vector.tensor_add(t1[:], t1[:], xr[:])
            dma(io["y"][bass.ts(half, 128), :], t1[:])


# ======================= host side =======================================

def _bf16(a):
    import ml_dtypes
    return np.ascontiguousarray(np.asarray(a).astype(ml_dtypes.bfloat16))


def _prep_inputs(hidden_states, temb, rope_cos, rope_sin, norm_w, norm_b,
                 qw, qb, kw, kb, vw, vb, q_rms_w, k_rms_w, mlp_w, mlp_b,
                 out_w, out_b):
    f32 = np.float32
    x = np.ascontiguousarray(np.asarray(hidden_states).reshape(S, D).astype(f32))
    xTb = _bf16(x.T)
    perm = np.concatenate([np.arange(0, DH, 2), np.arange(1, DH, 2)])
    cosH = np.asarray(rope_cos).astype(f32)[:, 0::2].T
    sinH = np.asarray(rope_sin).astype(f32)[:, 0::2].T
    cosP = np.concatenate([cosH, cosH], 0)
    sinP = np.concatenate([-sinH, sinH], 0)
    tembT = np.ascontiguousarray(
        np.asarray(temb).reshape(D).astype(f32).reshape(KT, 128).T)
    qwT_all = np.asarray(qw).T.astype(f32)
    kwT_all = np.asarray(kw).T.astype(f32)
    vwT_all = np.asarray(vw).T.astype(f32)
    mlpT_all = np.asarray(mlp_w).T.astype(f32)
    outT_all = np.asarray(out_w).T.astype(f32)
    normT_all = np.asarray(norm_w).T.astype(f32)
    cosP_b, sinP_b = _bf16(cosP), _bf16(sinP)

    in_maps = []
    for c in range(NC):
        heads = range(HPC * c, HPC * (c + 1))
        qk_cols = np.concatenate([h * DH + perm for h in heads])
        v_cols = np.concatenate([h * DH + np.arange(DH) for h in heads])
        ml_sl = slice(MPC * c, MPC * (c + 1))
        nm_sl = slice(NPC * c, NPC * (c + 1))
        out_rows = np.concatenate(
            [DQ * c + np.arange(DQ), D + MPC * c + np.arange(MPC)]
        )
        sres = np.concatenate(
            [S // RSC * i + RWC * c + np.arange(RWC)
             for i in range(RSC)]
        )
        # mlpT host layout [MT, 128, D]: H[m, p, t*128+f] = mlpT_all[t*128+p,
        # m*128+f] -> reshape/transpose
        mslab = mlpT_all[:, ml_sl].reshape(KT, 128, MT, 128)
        mslab = np.ascontiguousarray(
            mslab.transpose(2, 1, 0, 3).reshape(MT, 128, D))
        m = {
            "xTb": xTb,
            "xres": np.ascontiguousarray(x[sres]),
            "tembT": tembT,
            "outb_row": np.ascontiguousarray(
                np.asarray(out_b).astype(f32).reshape(1, D)),
            "normT": _bf16(normT_all[:, nm_sl]),
            "normb": np.ascontiguousarray(
                np.asarray(norm_b).astype(f32)[nm_sl].reshape(NJ, 128).T),
            "qwT": _bf16(qwT_all[:, qk_cols]),
            "kwT": _bf16(kwT_all[:, qk_cols]),
            "vwT": _bf16(vwT_all[:, v_cols]),
            "qb": np.ascontiguousarray(
                np.asarray(qb).astype(f32)[qk_cols].reshape(HPC, 128).T),
            "kb": np.ascontiguousarray(
                np.asarray(kb).astype(f32)[qk_cols].reshape(HPC, 128).T),
            "vb": np.ascontiguousarray(
                np.asarray(vb).astype(f32)[v_cols].reshape(HPC, 128).T),
            "qrw": np.ascontiguousarray(
                np.asarray(q_rms_w).astype(f32)[perm].reshape(128, 1)),
            "krw": np.ascontiguousarray(
                np.asarray(k_rms_w).astype(f32)[perm].reshape(128, 1)),
            "mlpT": _bf16(mslab),
            "mlpb": np.ascontiguousarray(
                np.asarray(mlp_b).astype(f32)[ml_sl].reshape(MT, 128).T),
            "outT": _bf16(outT_all[out_rows, :]),
            "cosP": cosP_b,
            "sinP": sinP_b,
        }
        in_maps.append(m)
    return in_maps


def run(inputs, debug=False, trace=False):
    nc = _build(debug=debug)
    in_maps = _prep_inputs(**inputs)
    res = run_bass_kernel_spmd(nc, in_maps, list(range(NC)), trace=trace)
    out = np.empty((S, D), np.float32)
    for c in range(NC):
        ys = res.results[c]["y"]
        for i in range(RSC):
            r0 = S // RSC * i + RWC * c
            out[r0 : r0 + RWC] = ys[RWC * i : RWC * (i + 1)]
    return out.reshape(B, S, D), res


def kernel(**inputs):
    out, _ = run(inputs)
    return out



# revision 8
# speedup vs baseline: 1.0678x; 1.0678x over previous
"""FluxSingleTransformerBlock on 8 trn2 NeuronCores.

Sharding: tensor-parallel over heads (3/core) and mlp_hidden (1536/core);
norm_w row-sharded with a tiny AllGather of emb; out-proj row-parallel with
chunked bf16 ReduceScatter overlapped with compute; final gate+residual on
each core's sequence shard; host concatenates disjoint shards.

v2: all activations SBUF-resident through qkv (no nx DRAM round trips),
quarter-granular LN stats with the emb matvec + AllGather overlapped under
them, MLP before QKV, fused (attn|mlp) activations spilled to DRAM and
re-read as 128-col stationary slices by the out-proj, out-proj weights
prefetched during attention, gate rows broadcast early so the tail is only
the last ReduceScatter chunk plus one residual block.
"""

import os
import sys

for _p in ("/opt/trn_rl_repo", "/root/.axon_site/_ro/trn_rl_repo"):
    if os.path.isdir(_p) and _p not in sys.path:
        sys.path.append(_p)

import numpy as np

import concourse.bass as bass
import concourse.bacc as bacc
import concourse.mybir as mybir
import concourse.tile as tile
import concourse.masks as masks
from concourse.bass_utils import run_bass_kernel_spmd

FP32 = mybir.dt.float32
BF16 = mybir.dt.bfloat16
AF = mybir.ActivationFunctionType
ALU = mybir.AluOpType

B, S, D, H, DH = 1, 2048, 3072, 24, 128
M = 4 * D  # 12288
NC = 8
HPC = H // NC            # heads per core = 3
DQ = HPC * DH            # q/k/v cols per core = 384
MPC = M // NC            # mlp rows per core = 1536
MT = MPC // 128          # 12
NPC = 3 * D // NC        # norm rows per core = 1152
NJ = NPC // 128          # 9
FPC = DQ + MPC           # fused contraction rows per core = 1920
FT = FPC // 128          # 15
KT = D // 128            # 24 d_in tiles
ST = S // 128            # 16 seq tiles
NCH = 4                  # seq chunks
QW = S // 4              # 512 stats quarter width
CW = S // NCH            # 512
W2 = S // 2              # 1024
RSC = 8                  # reduce-scatter chunks
RW = S // RSC            # 256 rows per rs chunk
RWC = RW // NC           # 32 rows per core per rs chunk
SSH = S // NC            # 256 final rows per core
EPS = 1e-6
ISQD = float(1.0 / np.sqrt(DH))

_CACHE = {}


def _build(debug=False):
    key = ("nc", debug)
    if key in _CACHE:
        return _CACHE[key]
    nc = bacc.Bacc("TRN2", target_bir_lowering=False, debug=False, num_devices=NC)

    def din(name, shape, dt=BF16):
        return nc.dram_tensor(name, list(shape), dt, kind="ExternalInput").ap()

    io = {
        "xTb": din("xTb", [D, S]),
        "xres": din("xres", [SSH, D], FP32),
        "tembT": din("tembT", [128, KT], FP32),
        "outb_row": din("outb_row", [1, D], FP32),
        "normT": din("normT", [D, NPC]),
        "normb": din("normb", [128, NJ], FP32),
        "qwT": din("qwT", [D, DQ]),
        "kwT": din("kwT", [D, DQ]),
        "vwT": din("vwT", [D, DQ]),
        "qb": din("qb", [128, HPC], FP32),
        "kb": din("kb", [128, HPC], FP32),
        "vb": din("vb", [128, HPC], FP32),
        "qrw": din("qrw", [128, 1], FP32),
        "krw": din("krw", [128, 1], FP32),
        "mlpT": din("mlpT", [MT, 128, D]),
        "mlpb": din("mlpb", [128, MT], FP32),
        "outT": din("outT", [FPC, D]),
        "cosP": din("cosP", [128, S]),
        "sinP": din("sinP", [128, S]),
        "y": nc.dram_tensor("y", [SSH, D], FP32, kind="ExternalOutput").ap(),
    }

    with tile.TileContext(nc) as tc:
        _emit(tc, io)
    nc.compile()
    _CACHE[key] = nc
    return nc


def _emit(tc, io):
    from contextlib import ExitStack

    with ExitStack() as ctx:
        _emit_body(ctx, tc, io)


def _emit_body(ctx, tc, io):
    nc = tc.nc

    def dma(out, in_):
        nc.sync.dma_start(out=out, in_=in_)

    # ---------- small constants ----------
    const = ctx.enter_context(tc.tile_pool(name="const", bufs=1))
    tembT_t = const.tile([128, KT], FP32)
    dma(tembT_t[:], io["tembT"][:, :])
    ones_b = const.tile([128, 128], BF16)
    nc.vector.memset(ones_b[:], 1.0)
    ident_b = const.tile([128, 128], BF16)
    masks.make_identity(nc, ident_b[:])
    ident_f = const.tile([128, 128], FP32)
    masks.make_identity(nc, ident_f[:])
    qrw_t = const.tile([128, 1], FP32)
    krw_t = const.tile([128, 1], FP32)
    dma(qrw_t[:], io["qrw"][:, :])
    dma(krw_t[:], io["krw"][:, :])
    qb_t = const.tile([128, HPC], FP32)
    kb_t = const.tile([128, HPC], FP32)
    vb_t = const.tile([128, HPC], FP32)
    dma(qb_t[:], io["qb"][:, :])
    dma(kb_t[:], io["kb"][:, :])
    dma(vb_t[:], io["vb"][:, :])
    mlpb_t = const.tile([128, MT], FP32)
    dma(mlpb_t[:], io["mlpb"][:, :])
    normb_t = const.tile([128, NJ], FP32)
    dma(normb_t[:], io["normb"][:, :])
    eps_t = const.tile([128, 1], FP32)
    nc.vector.memset(eps_t[:], EPS)

    adaln = ctx.enter_context(tc.tile_pool(name="adaln", bufs=1))
    shift_c = adaln.tile([128, KT], FP32)
    scale1_c = adaln.tile([128, KT], FP32)
    gate_c = adaln.tile([128, KT], FP32)

    dram = ctx.enter_context(tc.tile_pool(name="dram", bufs=1, space="DRAM"))
    fused_dram = dram.tile([FT, 128, S], BF16, tag="fusedd", name="fused_dram")

    # cos/sin survive through Q only
    csin_cm = tc.tile_pool(name="csin", bufs=1)
    csin = csin_cm.__enter__()
    cos_t = csin.tile([128, S], BF16)
    sin_t = csin.tile([128, S], BF16)
    dma(cos_t[:], io["cosP"][:, :])
    dma(sin_t[:], io["sinP"][:, :])

    # nx fully SBUF-resident: 24 x [128, S] bf16 (96 KiB/partition)
    nxp_cm = tc.tile_pool(name="nx", bufs=1)
    nxpool = nxp_cm.__enter__()
    nx = [nxpool.tile([128, S], BF16, tag=f"nx{t}", name=f"nx{t}")
          for t in range(KT)]

    # ---------- stats quarters + emb + AllGather, interleaved ----------
    stat_cm = tc.tile_pool(name="stat", bufs=1)
    statp = stat_cm.__enter__()
    RSTDb = statp.tile([128, S], BF16, tag="rstdb")
    C1 = statp.tile([128, S], FP32, tag="c1")
    xq_cm = tc.tile_pool(name="xq", bufs=1)
    xq = xq_cm.__enter__()
    sp_cm = tc.tile_pool(name="sp", bufs=2, space="PSUM")
    sp = sp_cm.__enter__()

    xtiles = {}          # (t, q) -> SBUF tile, alive until nx written
    stat_ps = {}         # q -> (sum_ps, ssq_ps)

    def emit_stats_quarter(q):
        qsl = bass.ds(q * QW, QW)
        sum_ps = sp.tile([128, QW], FP32, tag="sum")
        ssq_ps = sp.tile([128, QW], FP32, tag="ssq")
        for t in range(KT):
            xt = xq.tile([128, QW], BF16, tag="xin", bufs=30)
            dma(xt[:], io["xTb"][bass.ts(t, 128), qsl])
            xtiles[(t, q)] = xt
            sq = xq.tile([128, QW], BF16, tag="xsq", bufs=4)
            nc.scalar.activation(sq[:], xt[:], AF.Square)
            nc.tensor.matmul(sum_ps[:], ones_b[:], xt[:],
                             start=(t == 0), stop=(t == KT - 1))
            nc.tensor.matmul(ssq_ps[:], ones_b[:], sq[:],
                             start=(t == 0), stop=(t == KT - 1))
        stat_ps[q] = (sum_ps, ssq_ps)

    def emit_stats_finish(q):
        qsl = bass.ds(q * QW, QW)
        sum_ps, ssq_ps = stat_ps.pop(q)
        mu = statp.tile([128, QW], FP32, tag="mu", bufs=2)
        nc.scalar.activation(mu[:], sum_ps[:], AF.Copy, scale=1.0 / D)
        musq = statp.tile([128, QW], FP32, tag="musq", bufs=2)
        nc.vector.scalar_tensor_tensor(
            musq[:], mu[:], 0.0, mu[:], op0=ALU.bypass, op1=ALU.mult
        )
        nc.vector.tensor_scalar_add(musq[:], musq[:], -EPS)
        vare = statp.tile([128, QW], FP32, tag="vare", bufs=2)
        nc.vector.scalar_tensor_tensor(
            vare[:], ssq_ps[:], 1.0 / D, musq[:],
            op0=ALU.mult, op1=ALU.subtract,
        )
        rinv = statp.tile([128, QW], FP32, tag="rinv", bufs=2)
        rstd = statp.tile([128, QW], FP32, tag="rstd", bufs=2)
        nc.vector.reciprocal_approx_accurate(rinv[:], vare[:], rstd[:])
        nc.scalar.activation(rstd[:], rinv[:], AF.Sqrt)
        nc.vector.tensor_copy(RSTDb[:, qsl], rstd[:])
        nc.vector.tensor_mul(C1[:, qsl], mu[:], rstd[:])

    # stats quarter 0 first so the PE has work immediately
    emit_stats_quarter(0)

    # ---------- phase E: emb = silu(temb) @ norm_wT; AllGather ----------
    pe_cm = tc.tile_pool(name="pe", bufs=1)
    pe = pe_cm.__enter__()
    pep_cm = tc.tile_pool(name="pep", bufs=1, space="PSUM")
    pep = pep_cm.__enter__()
    p0w_cm = tc.tile_pool(name="p0w", bufs=4)
    p0w = p0w_cm.__enter__()
    st_t = pe.tile([128, KT], BF16)
    nc.scalar.activation(st_t[:], tembT_t[:], AF.Silu)
    # t-outer so normT slabs stream; per-t partials land in a fresh PSUM
    # tile (start=True clears has_written for the WHOLE bank, so columns
    # of one tile cannot carry interleaved accumulation groups) and are
    # summed on the DVE in SBUF.
    emb_sb = pe.tile([128, NJ], FP32)
    nc.vector.tensor_copy(emb_sb[:], normb_t[:])
    for t in range(KT):
        w = p0w.tile([128, NPC], BF16, tag="nslab")
        dma(w[:], io["normT"][bass.ts(t, 128), :])
        emb_ps = pep.tile([128, NJ], FP32, tag="embps", bufs=2)
        for j in range(NJ):
            nc.tensor.matmul(
                emb_ps[:, j : j + 1],
                w[:, bass.ts(j, 128)],
                st_t[:, t : t + 1],
                start=True,
                stop=True,
            )
        nc.vector.tensor_add(emb_sb[:], emb_sb[:], emb_ps[:])
    emb_loc = dram.tile([NJ, 128], FP32, tag="embloc")
    dma(emb_loc[:, :].rearrange("j p -> p j"), emb_sb[:])
    emb_all = dram.tile([NJ * NC, 128], FP32, tag="emball")
    nc.gpsimd.collective_compute(
        "AllGather",
        ALU.bypass,
        replica_groups=[list(range(NC))],
        ins=[emb_loc.opt()],
        outs=[emb_all.opt()],
    )
    dma(shift_c[:], emb_all[0:KT, :].rearrange("t p -> p t"))
    dma(scale1_c[:], emb_all[KT : 2 * KT, :].rearrange("t p -> p t"))
    dma(gate_c[:], emb_all[2 * KT : 3 * KT, :].rearrange("t p -> p t"))
    nc.vector.tensor_scalar_add(scale1_c[:], scale1_c[:], 1.0)

    # ---------- stats quarters 1-3 ----------
    emit_stats_finish(0)
    for q in range(1, 4):
        emit_stats_quarter(q)
        emit_stats_finish(q)
    p0w_cm.__exit__(None, None, None)
    pep_cm.__exit__(None, None, None)
    pe_cm.__exit__(None, None, None)

    # ---------- nx = (x*scale1_t)*rstd - (C1*scale1_t - shift_t) --------
    # q-major so x-tile buffers recycle quarter by quarter (no WAR stall);
    # the subtract runs on GpSimd to keep DVE throughput for fb/t2.
    nxt_cm = tc.tile_pool(name="nxt", bufs=1)
    nxt = nxt_cm.__enter__()
    for q in range(4):
        qsl = bass.ds(q * QW, QW)
        for t in range(KT):
            xt = xtiles.pop((t, q))
            fb = nxt.tile([128, QW], BF16, tag="fb", bufs=3)
            nc.vector.tensor_scalar(
                fb[:], C1[:, qsl], scale1_c[:, t : t + 1],
                shift_c[:, t : t + 1],
                op0=ALU.mult, op1=ALU.subtract,
            )
            t2 = nxt.tile([128, QW], BF16, tag="t2", bufs=3)
            nc.vector.scalar_tensor_tensor(
                t2[:], xt[:], scale1_c[:, t : t + 1], RSTDb[:, qsl],
                op0=ALU.mult, op1=ALU.mult,
            )
            nc.vector.tensor_sub(nx[t][:, qsl], t2[:], fb[:])
    nxt_cm.__exit__(None, None, None)
    sp_cm.__exit__(None, None, None)
    xq_cm.__exit__(None, None, None)
    stat_cm.__exit__(None, None, None)

    # ---------- phase M: mlp -> gelu -> fused_dram rows 3..14 ----------
    with (
        tc.tile_pool(name="p3w", bufs=2) as p3w,
        tc.tile_pool(name="p3g", bufs=2) as p3g,
        tc.tile_pool(name="p3p", bufs=2, space="PSUM") as p3p,
    ):
        for half in range(2):
            hsl = bass.ds(half * W2, W2)
            for m in range(MT):
                wm = p3w.tile([128, D], BF16, tag="wmlp")
                dma(wm[:], io["mlpT"][m, :, :])
                ps = p3p.tile([128, W2], FP32, tag="mlpps")
                for t in range(KT):
                    for c2 in range(2):
                        nc.tensor.matmul(
                            ps[:, bass.ts(c2, CW)], wm[:, bass.ts(t, 128)],
                            nx[t][:, bass.ds(half * W2 + c2 * CW, CW)],
                            start=(t == 0), stop=(t == KT - 1),
                        )
                gout = p3g.tile([128, W2], BF16, tag="gout")
                nc.scalar.activation(
                    gout[:], ps[:], AF.Gelu_apprx_tanh,
                    bias=mlpb_t[:, m : m + 1],
                )
                dma(fused_dram[HPC + m, :, hsl], gout[:])

    # ---------- phase Q: qkv + rmsnorm + rope (halves) ----------
    qkvp_cm = tc.tile_pool(name="qkv", bufs=1, side="right")
    qkvp = qkvp_cm.__enter__()
    qT = [qkvp.tile([128, S], BF16, tag=f"q{m}", name=f"q{m}") for m in range(HPC)]
    kT = [qkvp.tile([128, S], BF16, tag=f"k{m}", name=f"k{m}") for m in range(HPC)]
    vT = [qkvp.tile([128, S], BF16, tag=f"v{m}", name=f"v{m}") for m in range(HPC)]
    qkvT = [qT, kT, vT]
    with (
        tc.tile_pool(name="p2w", bufs=26) as p2w,
        tc.tile_pool(name="p2", bufs=2) as p2,
        tc.tile_pool(name="p2f", bufs=1) as p2f,
        tc.tile_pool(name="p2p", bufs=2, space="PSUM") as p2p,
        tc.tile_pool(name="p2p2", bufs=2, space="PSUM") as p2p2,
    ):
        for half in range(2):
            hsl = bass.ds(half * W2, W2)
            for pi, (wap, bt) in enumerate(
                ((io["qwT"], qb_t), (io["kwT"], kb_t), (io["vwT"], vb_t))
            ):
                wt = []
                for t in range(KT):
                    w = p2w.tile([128, DQ], BF16, tag="wqkv")
                    dma(w[:], wap[bass.ts(t, 128), :])
                    wt.append(w)
                for m in range(HPC):
                    ps = p2p.tile([128, W2], FP32, tag="qkvps")
                    for t in range(KT):
                        for c2 in range(2):
                            nc.tensor.matmul(
                                ps[:, bass.ts(c2, CW)],
                                wt[t][:, bass.ts(m, 128)],
                                nx[t][:, bass.ds(half * W2 + c2 * CW, CW)],
                                start=(t == 0), stop=(t == KT - 1),
                            )
                    if pi == 2:
                        nc.scalar.activation(
                            vT[m][:, hsl], ps[:], AF.Identity,
                            bias=bt[:, m : m + 1]
                        )
                        continue
                    qsb = p2.tile([128, W2], BF16, tag="qsb")
                    nc.scalar.activation(
                        qsb[:], ps[:], AF.Identity, bias=bt[:, m : m + 1]
                    )
                    sq = p2.tile([128, W2], BF16, tag="sq2")
                    nc.scalar.activation(
                        sq[:], ps[:], AF.Square, bias=bt[:, m : m + 1]
                    )
                    ssq = p2p2.tile([128, W2], FP32, tag="ssq2")
                    for c2 in range(2):
                        nc.tensor.matmul(
                            ssq[:, bass.ts(c2, CW)], ones_b[:],
                            sq[:, bass.ts(c2, CW)], start=True, stop=True,
                        )
                    vare = p2f.tile([128, W2], FP32, tag="vare2")
                    nc.scalar.activation(
                        vare[:], ssq[:], AF.Identity, bias=eps_t[:, 0:1],
                        scale=1.0 / DH,
                    )
                    rinv = p2f.tile([128, W2], FP32, tag="rinv2")
                    rst = p2f.tile([128, W2], FP32, tag="rst2")
                    nc.vector.reciprocal_approx_accurate(rinv[:], vare[:], rst[:])
                    nc.scalar.activation(rst[:], rinv[:], AF.Sqrt)
                    qn = p2.tile([128, W2], BF16, tag="qn")
                    rw = qrw_t if pi == 0 else krw_t
                    nc.vector.scalar_tensor_tensor(
                        qn[:], qsb[:], rw[:, 0:1], rst[:],
                        op0=ALU.mult, op1=ALU.mult,
                    )
                    # rope: out = qn*cos2 + swap(qn)*sin2, sin2 = [-s; s]
                    qsw = p2.tile([128, W2], BF16, tag="qsw")
                    nc.vector.tensor_copy(qsw[0:64, :], qn[64:128, :])
                    nc.vector.tensor_copy(qsw[64:128, :], qn[0:64, :])
                    te = p2.tile([128, W2], BF16, tag="te")
                    to = p2.tile([128, W2], BF16, tag="to")
                    nc.vector.tensor_mul(te[:], qn[:], cos_t[:, hsl])
                    nc.vector.tensor_mul(to[:], qsw[:], sin_t[:, hsl])
                    nc.vector.tensor_add(qkvT[pi][m][:, hsl], te[:], to[:])

    nxp_cm.__exit__(None, None, None)
    csin_cm.__exit__(None, None, None)

    # ---------- phase 6pre: broadcast gate/outb rows to [128, D] --------
    gatep_cm = tc.tile_pool(name="gatep", bufs=1)
    gatep = gatep_cm.__enter__()
    GATE = gatep.tile([128, D], BF16, tag="gateb")
    HB = gatep.tile([128, D], BF16, tag="hb")
    with (
        tc.tile_pool(name="p6a", bufs=1) as p6a,
        tc.tile_pool(name="p6ap", bufs=2, space="PSUM") as p6ap,
    ):
        grow_ps = p6ap.tile([KT, 128], FP32, tag="grow", bufs=1)
        nc.tensor.transpose(grow_ps[:], gate_c[:], ident_f[:])
        grow_sb = p6a.tile([KT, 128], FP32, tag="growt")
        nc.scalar.activation(grow_sb[:], grow_ps[:], AF.Copy)
        grow = p6a.tile([1, D], FP32, tag="growsb")
        dma(grow[0:1, :], grow_sb[:, :])
        ob_sb = p6a.tile([1, D], FP32, tag="obrow")
        dma(ob_sb[:], io["outb_row"][:, :])
        hrow = p6a.tile([1, D], FP32, tag="hrow")
        nc.vector.tensor_mul(hrow[:], grow[:], ob_sb[:])
        growb = p6a.tile([1, D], BF16, tag="growb")
        hrowb = p6a.tile([1, D], BF16, tag="hrowb")
        nc.vector.tensor_copy(growb[:], grow[:])
        nc.vector.tensor_copy(hrowb[:], hrow[:])
        for dc in range(D // CW):
            dsl = bass.ts(dc, CW)
            bps = p6ap.tile([128, CW], FP32, tag="bps")
            nc.tensor.matmul(
                bps[:], ones_b[0:1, :], growb[0:1, dsl], start=True, stop=True
            )
            nc.scalar.activation(GATE[:, dsl], bps[:], AF.Copy)
            bps2 = p6ap.tile([128, CW], FP32, tag="bps2")
            nc.tensor.matmul(
                bps2[:], ones_b[0:1, :], hrowb[0:1, dsl], start=True, stop=True
            )
            nc.scalar.activation(HB[:, dsl], bps2[:], AF.Copy)

    # residual work tiles opened early so phase-6 blocks can run during O
    p6b_cm = tc.tile_pool(name="p6b", bufs=1)
    p6b = p6b_cm.__enter__()

    # out-proj weights: prefetched during attention
    p5w_cm = tc.tile_pool(name="p5w", bufs=FT)
    p5w = p5w_cm.__enter__()
    wo = []
    for f in range(FT):
        w = p5w.tile([128, D], BF16, tag="wout")
        dma(w[:], io["outT"][bass.ts(f, 128), :])
        wo.append(w)

    # ---------- phase A: attention -> fused_dram rows 0..2 ----------
    with (
        tc.tile_pool(name="vt", bufs=HPC * ST, side="right") as vtp,
        tc.tile_pool(name="p4pt", bufs=20, side="right") as p4pt,
        tc.tile_pool(name="p4", bufs=1, side="right") as p4,
        tc.tile_pool(name="otmp", bufs=4, side="right") as otp,
    ):
        vts = {}
        with tc.tile_pool(name="p4vp", bufs=4, space="PSUM") as p4vp:
            for h in range(HPC):
                for kt in range(ST):
                    tp = p4vp.tile([128, 128], BF16, tag="vtp")
                    nc.tensor.transpose(
                        tp[:], vT[h][:, bass.ts(kt, 128)], ident_b[:]
                    )
                    vs = vtp.tile([128, 128], BF16, tag="vts")
                    nc.vector.tensor_copy(vs[:], tp[:])
                    vts[(h, kt)] = vs
        with (
            tc.tile_pool(name="p4p", bufs=3, space="PSUM") as p4p,
            tc.tile_pool(name="p4pa", bufs=2, space="PSUM") as p4pa,
        ):
            for h in range(HPC):
                for c in range(NCH):
                    csl = bass.ts(c, CW)
                    pts = []
                    for kt in range(ST):
                        sps = p4p.tile([128, CW], FP32, tag="sps")
                        nc.tensor.matmul(
                            sps[:], kT[h][:, bass.ts(kt, 128)], qT[h][:, csl],
                            start=True, stop=True,
                        )
                        pt = p4pt.tile([128, CW], BF16, tag="pt")
                        nc.scalar.activation(pt[:], sps[:], AF.Exp, scale=ISQD)
                        pts.append(pt)
                    ops = p4pa.tile([128, CW], FP32, tag="ops")
                    sms = p4pa.tile([128, CW], FP32, tag="sms")
                    for kt in range(ST):
                        nc.tensor.matmul(
                            ops[:], vts[(h, kt)][:], pts[kt][:],
                            start=(kt == 0), stop=(kt == ST - 1),
                        )
                    for kt in range(ST):
                        nc.tensor.matmul(
                            sms[:], ones_b[:], pts[kt][:],
                            start=(kt == 0), stop=(kt == ST - 1),
                        )
                    rec = p4.tile([128, CW], FP32, tag="rec")
                    scr = p4.tile([128, CW], FP32, tag="scr4")
                    nc.vector.reciprocal_approx_accurate(rec[:], sms[:], scr[:])
                    ot = otp.tile([128, CW], BF16, tag="ot")
                    nc.vector.tensor_mul(ot[:], ops[:], rec[:])
                    dma(fused_dram[h, :, csl], ot[:])

    qkvp_cm.__exit__(None, None, None)

    # ---------- phase O: out proj + 8-way chunked ReduceScatter ----------
    rs_out = []
    with (
        tc.tile_pool(name="fsl", bufs=2 * FT) as fsl,
        tc.tile_pool(name="p5", bufs=4) as p5,
        tc.tile_pool(name="p5p", bufs=2, space="PSUM") as p5p,
    ):
        for i in range(RSC):
            rs_in_i = dram.tile([RW, D], BF16, tag=f"rsin{i}", name=f"rsin{i}")
            rs_out_i = dram.tile([RWC, D], BF16, tag=f"rsout{i}",
                                 name=f"rsout{i}")
            rs_out.append(rs_out_i)
            for sblk in range(RW // 128):
                stile = i * (RW // 128) + sblk
                ssl = bass.ts(stile, 128)
                fs = []
                for f in range(FT):
                    fst = fsl.tile([128, 128], BF16, tag="fs")
                    dma(fst[:], fused_dram[f, :, ssl])
                    fs.append(fst)
                for dcp in range(D // (2 * CW)):
                    ps = p5p.tile([128, 2 * CW], FP32, tag="ops5")
                    for f in range(FT):
                        for c2 in range(2):
                            nc.tensor.matmul(
                                ps[:, bass.ts(c2, CW)], fs[f][:],
                                wo[f][:, bass.ds(dcp * 2 * CW + c2 * CW, CW)],
                                start=(f == 0), stop=(f == FT - 1),
                            )
                    osb = p5.tile([128, 2 * CW], BF16, tag="osb")
                    nc.scalar.activation(osb[:], ps[:], AF.Copy)
                    dma(rs_in_i[bass.ts(sblk, 128), bass.ts(dcp, 2 * CW)],
                        osb[:])
            nc.gpsimd.collective_compute(
                "ReduceScatter",
                ALU.add,
                replica_groups=[list(range(NC))],
                ins=[rs_in_i.opt()],
                outs=[rs_out_i.opt()],
            )
    p5w_cm.__exit__(None, None, None)

    # ---------- phase 6: gate + residual on own shard ----------
    for half in range(SSH // 128):
        sh = p6b.tile([128, D], BF16, tag="shard", bufs=1)
        for i in range(128 // RWC):
            blk = half * (128 // RWC) + i
            dma(sh[bass.ts(i, RWC), :], rs_out[blk][:, :])
        xr = p6b.tile([128, D], FP32, tag="xr", bufs=1)
        dma(xr[:], io["xres"][bass.ts(half, 128), :])
        gf = p6b.tile([128, D], BF16, tag="gf", bufs=1)
        nc.vector.tensor_mul(gf[:], sh[:], GATE[:])
        nc.vector.tensor_add(xr[:], xr[:], gf[:])
        nc.vector.tensor_add(xr[:], xr[:], HB[:])
        dma(io["y"][bass.ts(half, 128), :], xr[:])
    p6b_cm.__exit__(None, None, None)
    gatep_cm.__exit__(None, None, None)


# ======================= host side =======================================

def _bf16(a):
    import ml_dtypes
    return np.ascontiguousarray(np.asarray(a).astype(ml_dtypes.bfloat16))


def _prep_inputs(hidden_states, temb, rope_cos, rope_sin, norm_w, norm_b,
                 qw, qb, kw, kb, vw, vb, q_rms_w, k_rms_w, mlp_w, mlp_b,
                 out_w, out_b):
    f32 = np.float32
    x = np.ascontiguousarray(np.asarray(hidden_states).reshape(S, D).astype(f32))
    xTb = _bf16(x.T)
    perm = np.concatenate([np.arange(0, DH, 2), np.arange(1, DH, 2)])
    cosH = np.asarray(rope_cos).astype(f32)[:, 0::2].T
    sinH = np.asarray(rope_sin).astype(f32)[:, 0::2].T
    cosP = np.concatenate([cosH, cosH], 0)
    sinP = np.concatenate([-sinH, sinH], 0)
    tembT = np.ascontiguousarray(
        np.asarray(temb).reshape(D).astype(f32).reshape(KT, 128).T)
    qwT_all = np.asarray(qw).T.astype(f32)
    kwT_all = np.asarray(kw).T.astype(f32)
    vwT_all = np.asarray(vw).T.astype(f32)
    mlpT_all = np.asarray(mlp_w).T.astype(f32)
    outT_all = np.asarray(out_w).T.astype(f32)
    normT_all = np.asarray(norm_w).T.astype(f32)
    cosP_b, sinP_b = _bf16(cosP), _bf16(sinP)

    in_maps = []
    for c in range(NC):
        heads = range(HPC * c, HPC * (c + 1))
        qk_cols = np.concatenate([h * DH + perm for h in heads])
        v_cols = np.concatenate([h * DH + np.arange(DH) for h in heads])
        ml_sl = slice(MPC * c, MPC * (c + 1))
        nm_sl = slice(NPC * c, NPC * (c + 1))
        out_rows = np.concatenate(
            [DQ * c + np.arange(DQ), D + MPC * c + np.arange(MPC)]
        )
        sres = np.concatenate(
            [S // RSC * i + RWC * c + np.arange(RWC)
             for i in range(RSC)]
        )
        # mlpT host layout [MT, 128, D]: H[m, p, t*128+f] = mlpT_all[t*128+p,
        # m*128+f] -> reshape/transpose
        mslab = mlpT_all[:, ml_sl].reshape(KT, 128, MT, 128)
        mslab = np.ascontiguousarray(
            mslab.transpose(2, 1, 0, 3).reshape(MT, 128, D))
        m = {
            "xTb": xTb,
            "xres": np.ascontiguousarray(x[sres]),
            "tembT": tembT,
            "outb_row": np.ascontiguousarray(
                np.asarray(out_b).astype(f32).reshape(1, D)),
            "normT": _bf16(normT_all[:, nm_sl]),
            "normb": np.ascontiguousarray(
                np.asarray(norm_b).astype(f32)[nm_sl].reshape(NJ, 128).T),
            "qwT": _bf16(qwT_all[:, qk_cols]),
            "kwT": _bf16(kwT_all[:, qk_cols]),
            "vwT": _bf16(vwT_all[:, v_cols]),
            "qb": np.ascontiguousarray(
                np.asarray(qb).astype(f32)[qk_cols].reshape(HPC, 128).T),
            "kb": np.ascontiguousarray(
                np.asarray(kb).astype(f32)[qk_cols].reshape(HPC, 128).T),
            "vb": np.ascontiguousarray(
                np.asarray(vb).astype(f32)[v_cols].reshape(HPC, 128).T),
            "qrw": np.ascontiguousarray(
                np.asarray(q_rms_w).astype(f32)[perm].reshape(128, 1)),
            "krw": np.ascontiguousarray(
                np.asarray(k_rms_w).astype(f32)[perm].reshape(128, 1)),
            "mlpT": _bf16(mslab),
            "mlpb": np.ascontiguousarray(
                np.asarray(mlp_b).astype(f32)[ml_sl].reshape(MT, 128).T),
            "outT": _bf16(outT_all[out_rows, :]),
            "cosP": cosP_b,
            "sinP": sinP_b,
        }
        in_maps.append(m)
    return in_maps


def run(inputs, debug=False, trace=False):
    nc = _build(debug=debug)
    in_maps = _prep_inputs(**inputs)
    res = run_bass_kernel_spmd(nc, in_maps, list(range(NC)), trace=trace)
    out = np.empty((S, D), np.float32)
    for c in range(NC):
        ys = res.results[c]["y"]
        for i in range(RSC):
            r0 = S // RSC * i + RWC * c
            out[r0 : r0 + RWC] = ys[RWC * i : RWC * (i + 1)]
    return out.reshape(B, S, D), res


def kernel(**inputs):
    out, _ = run(inputs)
    return out


# revision 11
# speedup vs baseline: 1.1246x; 1.0532x over previous
"""FluxSingleTransformerBlock on 8 trn2 NeuronCores.

Sharding: tensor-parallel over heads (3/core) and mlp_hidden (1536/core);
norm_w row-sharded with a tiny AllGather of emb; out-proj row-parallel with
chunked bf16 ReduceScatter overlapped with compute; final gate+residual on
each core's sequence shard; host concatenates disjoint shards.

v2: all activations SBUF-resident through qkv (no nx DRAM round trips),
quarter-granular LN stats with the emb matvec + AllGather overlapped under
them, MLP before QKV, fused (attn|mlp) activations spilled to DRAM and
re-read as 128-col stationary slices by the out-proj, out-proj weights
prefetched during attention, gate rows broadcast early so the tail is only
the last ReduceScatter chunk plus one residual block.
"""

import os
import sys

for _p in ("/opt/trn_rl_repo", "/root/.axon_site/_ro/trn_rl_repo"):
    if os.path.isdir(_p) and _p not in sys.path:
        sys.path.append(_p)

import numpy as np

import concourse.bass as bass
import concourse.bacc as bacc
import concourse.mybir as mybir
import concourse.tile as tile
import concourse.masks as masks
from concourse.bass_utils import run_bass_kernel_spmd

FP32 = mybir.dt.float32
BF16 = mybir.dt.bfloat16
AF = mybir.ActivationFunctionType
ALU = mybir.AluOpType

B, S, D, H, DH = 1, 2048, 3072, 24, 128
M = 4 * D  # 12288
NC = 8
HPC = H // NC            # heads per core = 3
DQ = HPC * DH            # q/k/v cols per core = 384
MPC = M // NC            # mlp rows per core = 1536
MT = MPC // 128          # 12
NPC = 3 * D // NC        # norm rows per core = 1152
NJ = NPC // 128          # 9
FPC = DQ + MPC           # fused contraction rows per core = 1920
FT = FPC // 128          # 15
KT = D // 128            # 24 d_in tiles
ST = S // 128            # 16 seq tiles
NCH = 4                  # seq chunks
QW = S // 4              # 512 stats quarter width
CW = S // NCH            # 512
W2 = S // 2              # 1024
RSC = 8                  # reduce-scatter chunks
RW = S // RSC            # 256 rows per rs chunk
RWC = RW // NC           # 32 rows per core per rs chunk
SSH = S // NC            # 256 final rows per core
EPS = 1e-6
ISQD = float(1.0 / np.sqrt(DH))

_CACHE = {}


def _build(debug=False):
    key = ("nc", debug)
    if key in _CACHE:
        return _CACHE[key]
    nc = bacc.Bacc("TRN2", target_bir_lowering=False, debug=False, num_devices=NC)

    def din(name, shape, dt=BF16):
        return nc.dram_tensor(name, list(shape), dt, kind="ExternalInput").ap()

    io = {
        "xTb": din("xTb", [D, S]),
        "xres": din("xres", [SSH, D], FP32),
        "tembT": din("tembT", [128, KT], FP32),
        "outb_row": din("outb_row", [1, D], FP32),
        "normT": din("normT", [D, NPC]),
        "normb": din("normb", [128, NJ], FP32),
        "qwT": din("qwT", [D, DQ]),
        "kwT": din("kwT", [D, DQ]),
        "vwT": din("vwT", [D, DQ]),
        "qb": din("qb", [128, HPC], FP32),
        "kb": din("kb", [128, HPC], FP32),
        "vb": din("vb", [128, HPC], FP32),
        "qrw": din("qrw", [128, 1], FP32),
        "krw": din("krw", [128, 1], FP32),
        "mlpT": din("mlpT", [MT, 128, D]),
        "mlpb": din("mlpb", [128, MT], FP32),
        "outT": din("outT", [FPC, D]),
        "cosP": din("cosP", [128, S]),
        "sinP": din("sinP", [128, S]),
        "y": nc.dram_tensor("y", [SSH, D], FP32, kind="ExternalOutput").ap(),
    }

    with tile.TileContext(nc) as tc:
        _emit(tc, io)
    nc.compile()
    _CACHE[key] = nc
    return nc


def _emit(tc, io):
    from contextlib import ExitStack

    with ExitStack() as ctx:
        _emit_body(ctx, tc, io)


def _emit_body(ctx, tc, io):
    nc = tc.nc

    def dma(out, in_):
        nc.sync.dma_start(out=out, in_=in_)

    # ---------- small constants ----------
    const = ctx.enter_context(tc.tile_pool(name="const", bufs=1))
    tembT_t = const.tile([128, KT], FP32)
    dma(tembT_t[:], io["tembT"][:, :])
    ones_b = const.tile([128, 128], BF16)
    nc.vector.memset(ones_b[:], 1.0)
    ident_b = const.tile([128, 128], BF16)
    masks.make_identity(nc, ident_b[:])
    ident_f = const.tile([128, 128], FP32)
    masks.make_identity(nc, ident_f[:])
    qrw_t = const.tile([128, 1], FP32)
    krw_t = const.tile([128, 1], FP32)
    dma(qrw_t[:], io["qrw"][:, :])
    dma(krw_t[:], io["krw"][:, :])
    qb_t = const.tile([128, HPC], FP32)
    kb_t = const.tile([128, HPC], FP32)
    vb_t = const.tile([128, HPC], FP32)
    dma(qb_t[:], io["qb"][:, :])
    dma(kb_t[:], io["kb"][:, :])
    dma(vb_t[:], io["vb"][:, :])
    mlpb_t = const.tile([128, MT], FP32)
    dma(mlpb_t[:], io["mlpb"][:, :])
    normb_t = const.tile([128, NJ], FP32)
    dma(normb_t[:], io["normb"][:, :])
    eps_t = const.tile([128, 1], FP32)
    nc.vector.memset(eps_t[:], EPS)

    adaln = ctx.enter_context(tc.tile_pool(name="adaln", bufs=1))
    shift_c = adaln.tile([128, KT], FP32)
    scale1_c = adaln.tile([128, KT], FP32)
    gate_c = adaln.tile([128, KT], FP32)

    dram = ctx.enter_context(tc.tile_pool(name="dram", bufs=1, space="DRAM"))
    fused_dram = dram.tile([FT, 128, S], BF16, tag="fusedd", name="fused_dram")

    # cos/sin survive through Q only
    csin_cm = tc.tile_pool(name="csin", bufs=1)
    csin = csin_cm.__enter__()
    cos_t = csin.tile([128, S], BF16)
    sin_t = csin.tile([128, S], BF16)
    dma(cos_t[:], io["cosP"][:, :])
    dma(sin_t[:], io["sinP"][:, :])

    # nx fully SBUF-resident: 24 x [128, S] bf16 (96 KiB/partition)
    nxp_cm = tc.tile_pool(name="nx", bufs=1)
    nxpool = nxp_cm.__enter__()
    nx = [nxpool.tile([128, S], BF16, tag=f"nx{t}", name=f"nx{t}")
          for t in range(KT)]

    # ---------- stats quarters + emb + AllGather, interleaved ----------
    stat_cm = tc.tile_pool(name="stat", bufs=1)
    statp = stat_cm.__enter__()
    RSTDb = statp.tile([128, S], BF16, tag="rstdb")
    C1 = statp.tile([128, S], BF16, tag="c1")
    xq_cm = tc.tile_pool(name="xq", bufs=1)
    xq = xq_cm.__enter__()
    sp_cm = tc.tile_pool(name="sp", bufs=2, space="PSUM")
    sp = sp_cm.__enter__()

    xtiles = {}          # (t, q) -> SBUF tile, alive until nx written
    stat_ps = {}         # q -> (sum_ps, ssq_ps)

    def emit_stats_quarter(q):
        qsl = bass.ds(q * QW, QW)
        sum_ps = sp.tile([128, QW], FP32, tag="sum")
        ssq_ps = sp.tile([128, QW], FP32, tag="ssq")
        for t in range(KT):
            xt = xq.tile([128, QW], BF16, tag="xin", bufs=30)
            dma(xt[:], io["xTb"][bass.ts(t, 128), qsl])
            xtiles[(t, q)] = xt
            sq = xq.tile([128, QW], BF16, tag="xsq", bufs=4)
            nc.scalar.activation(sq[:], xt[:], AF.Square)
            nc.tensor.matmul(sum_ps[:], ones_b[:], xt[:],
                             start=(t == 0), stop=(t == KT - 1))
            nc.tensor.matmul(ssq_ps[:], ones_b[:], sq[:],
                             start=(t == 0), stop=(t == KT - 1))
        stat_ps[q] = (sum_ps, ssq_ps)

    def emit_stats_finish(q):
        qsl = bass.ds(q * QW, QW)
        sum_ps, ssq_ps = stat_ps.pop(q)
        mu = statp.tile([128, QW], FP32, tag="mu", bufs=2)
        nc.scalar.activation(mu[:], sum_ps[:], AF.Copy, scale=1.0 / D)
        musq = statp.tile([128, QW], FP32, tag="musq", bufs=2)
        nc.vector.scalar_tensor_tensor(
            musq[:], mu[:], 0.0, mu[:], op0=ALU.bypass, op1=ALU.mult
        )
        nc.vector.tensor_scalar_add(musq[:], musq[:], -EPS)
        vare = statp.tile([128, QW], FP32, tag="vare", bufs=2)
        nc.vector.scalar_tensor_tensor(
            vare[:], ssq_ps[:], 1.0 / D, musq[:],
            op0=ALU.mult, op1=ALU.subtract,
        )
        rinv = statp.tile([128, QW], FP32, tag="rinv", bufs=2)
        rstd = statp.tile([128, QW], FP32, tag="rstd", bufs=2)
        nc.vector.reciprocal_approx_accurate(rinv[:], vare[:], rstd[:])
        nc.scalar.activation(rstd[:], rinv[:], AF.Sqrt)
        nc.vector.tensor_copy(RSTDb[:, qsl], rstd[:])
        nc.vector.tensor_mul(C1[:, qsl], mu[:], rstd[:])
        # z = x*rstd - C1 into the nx tiles: AG-independent, frees x bufs
        # early (bf16 2x-mode TT ops)
        for t in range(KT):
            xt = xtiles.pop((t, q))
            nsl = nx[t][:, qsl]
            nc.vector.tensor_mul(nsl, xt[:], RSTDb[:, qsl])
            nc.vector.tensor_sub(nsl, nsl, C1[:, qsl])

    # stats quarter 0 first so the PE has work immediately
    emit_stats_quarter(0)

    # ---------- phase E: emb = silu(temb) @ norm_wT; AllGather ----------
    pe_cm = tc.tile_pool(name="pe", bufs=1)
    pe = pe_cm.__enter__()
    pep_cm = tc.tile_pool(name="pep", bufs=1, space="PSUM")
    pep = pep_cm.__enter__()
    p0w_cm = tc.tile_pool(name="p0w", bufs=4)
    p0w = p0w_cm.__enter__()
    st_t = pe.tile([128, KT], BF16)
    nc.scalar.activation(st_t[:], tembT_t[:], AF.Silu)
    # t-outer so normT slabs stream; per-t partials land in a fresh PSUM
    # tile (start=True clears has_written for the WHOLE bank, so columns
    # of one tile cannot carry interleaved accumulation groups) and are
    # summed on the DVE in SBUF.
    emb_sb = pe.tile([128, NJ], FP32)
    nc.vector.tensor_copy(emb_sb[:], normb_t[:])
    for t in range(KT):
        w = p0w.tile([128, NPC], BF16, tag="nslab")
        dma(w[:], io["normT"][bass.ts(t, 128), :])
        emb_ps = pep.tile([128, NJ], FP32, tag="embps", bufs=2)
        for j in range(NJ):
            nc.tensor.matmul(
                emb_ps[:, j : j + 1],
                w[:, bass.ts(j, 128)],
                st_t[:, t : t + 1],
                start=True,
                stop=True,
            )
        nc.vector.tensor_add(emb_sb[:], emb_sb[:], emb_ps[:])
    emb_loc = dram.tile([NJ, 128], FP32, tag="embloc")
    dma(emb_loc[:, :].rearrange("j p -> p j"), emb_sb[:])
    emb_all = dram.tile([NJ * NC, 128], FP32, tag="emball")
    nc.gpsimd.collective_compute(
        "AllGather",
        ALU.bypass,
        replica_groups=[list(range(NC))],
        ins=[emb_loc.opt()],
        outs=[emb_all.opt()],
    )
    dma(shift_c[:], emb_all[0:KT, :].rearrange("t p -> p t"))
    dma(scale1_c[:], emb_all[KT : 2 * KT, :].rearrange("t p -> p t"))
    dma(gate_c[:], emb_all[2 * KT : 3 * KT, :].rearrange("t p -> p t"))
    nc.vector.tensor_scalar_add(scale1_c[:], scale1_c[:], 1.0)

    # ---------- stats quarters 1-3 ----------
    emit_stats_finish(0)
    for q in range(1, 4):
        emit_stats_quarter(q)
        emit_stats_finish(q)
    p0w_cm.__exit__(None, None, None)
    pep_cm.__exit__(None, None, None)
    pe_cm.__exit__(None, None, None)

    # ---------- nx = z * scale1_t + shift_t (one 4x-mode op per tile) ----
    for t in range(KT):
        nc.vector.tensor_scalar(
            nx[t][:], nx[t][:], scale1_c[:, t : t + 1],
            shift_c[:, t : t + 1],
            op0=ALU.mult, op1=ALU.add,
        )
    sp_cm.__exit__(None, None, None)
    xq_cm.__exit__(None, None, None)
    stat_cm.__exit__(None, None, None)

    # ---------- phase M: mlp -> gelu -> fused_dram rows 3..14 ----------
    with (
        tc.tile_pool(name="p3w", bufs=2) as p3w,
        tc.tile_pool(name="p3g", bufs=2) as p3g,
        tc.tile_pool(name="p3p", bufs=2, space="PSUM") as p3p,
    ):
        for half in range(2):
            hsl = bass.ds(half * W2, W2)
            for m in range(MT):
                wm = p3w.tile([128, D], BF16, tag="wmlp")
                dma(wm[:], io["mlpT"][m, :, :])
                ps = p3p.tile([128, W2], FP32, tag="mlpps")
                for t in range(KT):
                    for c2 in range(2):
                        nc.tensor.matmul(
                            ps[:, bass.ts(c2, CW)], wm[:, bass.ts(t, 128)],
                            nx[t][:, bass.ds(half * W2 + c2 * CW, CW)],
                            start=(t == 0), stop=(t == KT - 1),
                        )
                gout = p3g.tile([128, W2], BF16, tag="gout")
                nc.scalar.activation(
                    gout[:], ps[:], AF.Gelu_apprx_tanh,
                    bias=mlpb_t[:, m : m + 1],
                )
                dma(fused_dram[HPC + m, :, hsl], gout[:])

    # ---------- phase Q: qkv + rmsnorm + rope (halves) ----------
    qkvp_cm = tc.tile_pool(name="qkv", bufs=1, side="right")
    qkvp = qkvp_cm.__enter__()
    qT = [qkvp.tile([128, S], BF16, tag=f"q{m}", name=f"q{m}") for m in range(HPC)]
    kT = [qkvp.tile([128, S], BF16, tag=f"k{m}", name=f"k{m}") for m in range(HPC)]
    vT = [qkvp.tile([128, S], BF16, tag=f"v{m}", name=f"v{m}") for m in range(HPC)]
    qkvT = [qT, kT, vT]
    with (
        tc.tile_pool(name="p2w", bufs=26) as p2w,
        tc.tile_pool(name="p2", bufs=2) as p2,
        tc.tile_pool(name="p2f", bufs=1) as p2f,
        tc.tile_pool(name="p2p", bufs=2, space="PSUM") as p2p,
        tc.tile_pool(name="p2p2", bufs=2, space="PSUM") as p2p2,
    ):
        for half in range(2):
            hsl = bass.ds(half * W2, W2)
            for pi, (wap, bt) in enumerate(
                ((io["qwT"], qb_t), (io["kwT"], kb_t), (io["vwT"], vb_t))
            ):
                wt = []
                for t in range(KT):
                    w = p2w.tile([128, DQ], BF16, tag="wqkv")
                    dma(w[:], wap[bass.ts(t, 128), :])
                    wt.append(w)
                for m in range(HPC):
                    ps = p2p.tile([128, W2], FP32, tag="qkvps")
                    for t in range(KT):
                        for c2 in range(2):
                            nc.tensor.matmul(
                                ps[:, bass.ts(c2, CW)],
                                wt[t][:, bass.ts(m, 128)],
                                nx[t][:, bass.ds(half * W2 + c2 * CW, CW)],
                                start=(t == 0), stop=(t == KT - 1),
                            )
                    if pi == 2:
                        nc.scalar.activation(
                            vT[m][:, hsl], ps[:], AF.Identity,
                            bias=bt[:, m : m + 1]
                        )
                        continue
                    qsb = p2.tile([128, W2], BF16, tag="qsb")
                    nc.scalar.activation(
                        qsb[:], ps[:], AF.Identity, bias=bt[:, m : m + 1]
                    )
                    sq = p2.tile([128, W2], BF16, tag="sq2")
                    nc.scalar.activation(
                        sq[:], ps[:], AF.Square, bias=bt[:, m : m + 1]
                    )
                    ssq = p2p2.tile([128, W2], FP32, tag="ssq2")
                    for c2 in range(2):
                        nc.tensor.matmul(
                            ssq[:, bass.ts(c2, CW)], ones_b[:],
                            sq[:, bass.ts(c2, CW)], start=True, stop=True,
                        )
                    vare = p2f.tile([128, W2], FP32, tag="vare2")
                    nc.scalar.activation(
                        vare[:], ssq[:], AF.Identity, bias=eps_t[:, 0:1],
                        scale=1.0 / DH,
                    )
                    rinv = p2f.tile([128, W2], FP32, tag="rinv2")
                    rst = p2f.tile([128, W2], FP32, tag="rst2")
                    nc.vector.reciprocal_approx_accurate(rinv[:], vare[:], rst[:])
                    nc.scalar.activation(rst[:], rinv[:], AF.Sqrt)
                    qn = p2.tile([128, W2], BF16, tag="qn")
                    rw = qrw_t if pi == 0 else krw_t
                    nc.vector.scalar_tensor_tensor(
                        qn[:], qsb[:], rw[:, 0:1], rst[:],
                        op0=ALU.mult, op1=ALU.mult,
                    )
                    # rope: out = qn*cos2 + swap(qn)*sin2, sin2 = [-s; s]
                    qsw = p2.tile([128, W2], BF16, tag="qsw")
                    nc.vector.tensor_copy(qsw[0:64, :], qn[64:128, :])
                    nc.vector.tensor_copy(qsw[64:128, :], qn[0:64, :])
                    te = p2.tile([128, W2], BF16, tag="te")
                    to = p2.tile([128, W2], BF16, tag="to")
                    nc.vector.tensor_mul(te[:], qn[:], cos_t[:, hsl])
                    nc.vector.tensor_mul(to[:], qsw[:], sin_t[:, hsl])
                    nc.vector.tensor_add(qkvT[pi][m][:, hsl], te[:], to[:])

    nxp_cm.__exit__(None, None, None)
    csin_cm.__exit__(None, None, None)

    # ---------- phase 6pre: broadcast gate/outb rows to [128, D] --------
    gatep_cm = tc.tile_pool(name="gatep", bufs=1)
    gatep = gatep_cm.__enter__()
    GATE = gatep.tile([128, D], BF16, tag="gateb")
    HB = gatep.tile([128, D], BF16, tag="hb")
    with (
        tc.tile_pool(name="p6a", bufs=1) as p6a,
        tc.tile_pool(name="p6ap", bufs=2, space="PSUM") as p6ap,
    ):
        grow_ps = p6ap.tile([KT, 128], FP32, tag="grow", bufs=1)
        nc.tensor.transpose(grow_ps[:], gate_c[:], ident_f[:])
        grow_sb = p6a.tile([KT, 128], FP32, tag="growt")
        nc.scalar.activation(grow_sb[:], grow_ps[:], AF.Copy)
        grow = p6a.tile([1, D], FP32, tag="growsb")
        dma(grow[0:1, :], grow_sb[:, :])
        ob_sb = p6a.tile([1, D], FP32, tag="obrow")
        dma(ob_sb[:], io["outb_row"][:, :])
        hrow = p6a.tile([1, D], FP32, tag="hrow")
        nc.vector.tensor_mul(hrow[:], grow[:], ob_sb[:])
        growb = p6a.tile([1, D], BF16, tag="growb")
        hrowb = p6a.tile([1, D], BF16, tag="hrowb")
        nc.vector.tensor_copy(growb[:], grow[:])
        nc.vector.tensor_copy(hrowb[:], hrow[:])
        for dc in range(D // CW):
            dsl = bass.ts(dc, CW)
            bps = p6ap.tile([128, CW], FP32, tag="bps")
            nc.tensor.matmul(
                bps[:], ones_b[0:1, :], growb[0:1, dsl], start=True, stop=True
            )
            nc.scalar.activation(GATE[:, dsl], bps[:], AF.Copy)
            bps2 = p6ap.tile([128, CW], FP32, tag="bps2")
            nc.tensor.matmul(
                bps2[:], ones_b[0:1, :], hrowb[0:1, dsl], start=True, stop=True
            )
            nc.scalar.activation(HB[:, dsl], bps2[:], AF.Copy)

    # residual work tiles opened early so phase-6 blocks can run during O
    p6b_cm = tc.tile_pool(name="p6b", bufs=1)
    p6b = p6b_cm.__enter__()

    # out-proj weights: prefetched during attention
    p5w_cm = tc.tile_pool(name="p5w", bufs=FT)
    p5w = p5w_cm.__enter__()
    wo = []
    for f in range(FT):
        w = p5w.tile([128, D], BF16, tag="wout")
        dma(w[:], io["outT"][bass.ts(f, 128), :])
        wo.append(w)

    # ---------- phase A: attention -> fused_dram rows 0..2 ----------
    with (
        tc.tile_pool(name="vt", bufs=HPC * ST, side="right") as vtp,
        tc.tile_pool(name="p4pt", bufs=20, side="right") as p4pt,
        tc.tile_pool(name="p4", bufs=1, side="right") as p4,
        tc.tile_pool(name="otmp", bufs=4, side="right") as otp,
    ):
        vts = {}
        with tc.tile_pool(name="p4vp", bufs=4, space="PSUM") as p4vp:
            for h in range(HPC):
                for kt in range(ST):
                    tp = p4vp.tile([128, 128], BF16, tag="vtp")
                    nc.tensor.transpose(
                        tp[:], vT[h][:, bass.ts(kt, 128)], ident_b[:]
                    )
                    vs = vtp.tile([128, 128], BF16, tag="vts")
                    nc.vector.tensor_copy(vs[:], tp[:])
                    vts[(h, kt)] = vs
        with (
            tc.tile_pool(name="p4p", bufs=3, space="PSUM") as p4p,
            tc.tile_pool(name="p4pa", bufs=2, space="PSUM") as p4pa,
        ):
            for h in range(HPC):
                for c in range(NCH):
                    csl = bass.ts(c, CW)
                    pts = []
                    for kt in range(ST):
                        sps = p4p.tile([128, CW], FP32, tag="sps")
                        nc.tensor.matmul(
                            sps[:], kT[h][:, bass.ts(kt, 128)], qT[h][:, csl],
                            start=True, stop=True,
                        )
                        pt = p4pt.tile([128, CW], BF16, tag="pt")
                        nc.scalar.activation(pt[:], sps[:], AF.Exp, scale=ISQD)
                        pts.append(pt)
                    ops = p4pa.tile([128, CW], FP32, tag="ops")
                    sms = p4pa.tile([128, CW], FP32, tag="sms")
                    for kt in range(ST):
                        nc.tensor.matmul(
                            ops[:], vts[(h, kt)][:], pts[kt][:],
                            start=(kt == 0), stop=(kt == ST - 1),
                        )
                    for kt in range(ST):
                        nc.tensor.matmul(
                            sms[:], ones_b[:], pts[kt][:],
                            start=(kt == 0), stop=(kt == ST - 1),
                        )
                    rec = p4.tile([128, CW], FP32, tag="rec")
                    scr = p4.tile([128, CW], FP32, tag="scr4")
                    nc.vector.reciprocal_approx_accurate(rec[:], sms[:], scr[:])
                    ot = otp.tile([128, CW], BF16, tag="ot")
                    nc.vector.tensor_mul(ot[:], ops[:], rec[:])
                    dma(fused_dram[h, :, csl], ot[:])

    qkvp_cm.__exit__(None, None, None)

    # ---------- phase O: out proj + 8-way chunked ReduceScatter ----------
    rs_out = []
    with (
        tc.tile_pool(name="fsl", bufs=2 * FT) as fsl,
        tc.tile_pool(name="p5", bufs=4) as p5,
        tc.tile_pool(name="p5p", bufs=2, space="PSUM") as p5p,
    ):
        for i in range(RSC):
            rs_in_i = dram.tile([RW, D], BF16, tag=f"rsin{i}", name=f"rsin{i}")
            rs_out_i = dram.tile([RWC, D], BF16, tag=f"rsout{i}",
                                 name=f"rsout{i}")
            rs_out.append(rs_out_i)
            for sblk in range(RW // 128):
                stile = i * (RW // 128) + sblk
                ssl = bass.ts(stile, 128)
                fs = []
                for f in range(FT):
                    fst = fsl.tile([128, 128], BF16, tag="fs")
                    dma(fst[:], fused_dram[f, :, ssl])
                    fs.append(fst)
                for dcp in range(D // (2 * CW)):
                    ps = p5p.tile([128, 2 * CW], FP32, tag="ops5")
                    for f in range(FT):
                        for c2 in range(2):
                            nc.tensor.matmul(
                                ps[:, bass.ts(c2, CW)], fs[f][:],
                                wo[f][:, bass.ds(dcp * 2 * CW + c2 * CW, CW)],
                                start=(f == 0), stop=(f == FT - 1),
                            )
                    osb = p5.tile([128, 2 * CW], BF16, tag="osb")
                    nc.scalar.activation(osb[:], ps[:], AF.Copy)
                    dma(rs_in_i[bass.ts(sblk, 128), bass.ts(dcp, 2 * CW)],
                        osb[:])
            nc.gpsimd.collective_compute(
                "ReduceScatter",
                ALU.add,
                replica_groups=[list(range(NC))],
                ins=[rs_in_i.opt()],
                outs=[rs_out_i.opt()],
            )
    p5w_cm.__exit__(None, None, None)

    # ---------- phase 6: gate + residual on own shard ----------
    for half in range(SSH // 128):
        sh = p6b.tile([128, D], BF16, tag="shard", bufs=1)
        for i in range(128 // RWC):
            blk = half * (128 // RWC) + i
            dma(sh[bass.ts(i, RWC), :], rs_out[blk][:, :])
        xr = p6b.tile([128, D], FP32, tag="xr", bufs=1)
        dma(xr[:], io["xres"][bass.ts(half, 128), :])
        gf = p6b.tile([128, D], BF16, tag="gf", bufs=1)
        nc.vector.tensor_mul(gf[:], sh[:], GATE[:])
        nc.vector.tensor_add(xr[:], xr[:], gf[:])
        nc.vector.tensor_add(xr[:], xr[:], HB[:])
        dma(io["y"][bass.ts(half, 128), :], xr[:])
    p6b_cm.__exit__(None, None, None)
    gatep_cm.__exit__(None, None, None)


# ======================= host side =======================================

def _bf16(a):
    import ml_dtypes
    return np.ascontiguousarray(np.asarray(a).astype(ml_dtypes.bfloat16))


def _prep_inputs(hidden_states, temb, rope_cos, rope_sin, norm_w, norm_b,
                 qw, qb, kw, kb, vw, vb, q_rms_w, k_rms_w, mlp_w, mlp_b,
                 out_w, out_b):
    f32 = np.float32
    x = np.ascontiguousarray(np.asarray(hidden_states).reshape(S, D).astype(f32))
    xTb = _bf16(x.T)
    perm = np.concatenate([np.arange(0, DH, 2), np.arange(1, DH, 2)])
    cosH = np.asarray(rope_cos).astype(f32)[:, 0::2].T
    sinH = np.asarray(rope_sin).astype(f32)[:, 0::2].T
    cosP = np.concatenate([cosH, cosH], 0)
    sinP = np.concatenate([-sinH, sinH], 0)
    tembT = np.ascontiguousarray(
        np.asarray(temb).reshape(D).astype(f32).reshape(KT, 128).T)
    qwT_all = np.asarray(qw).T.astype(f32)
    kwT_all = np.asarray(kw).T.astype(f32)
    vwT_all = np.asarray(vw).T.astype(f32)
    mlpT_all = np.asarray(mlp_w).T.astype(f32)
    outT_all = np.asarray(out_w).T.astype(f32)
    normT_all = np.asarray(norm_w).T.astype(f32)
    cosP_b, sinP_b = _bf16(cosP), _bf16(sinP)

    in_maps = []
    for c in range(NC):
        heads = range(HPC * c, HPC * (c + 1))
        qk_cols = np.concatenate([h * DH + perm for h in heads])
        v_cols = np.concatenate([h * DH + np.arange(DH) for h in heads])
        ml_sl = slice(MPC * c, MPC * (c + 1))
        nm_sl = slice(NPC * c, NPC * (c + 1))
        out_rows = np.concatenate(
            [DQ * c + np.arange(DQ), D + MPC * c + np.arange(MPC)]
        )
        sres = np.concatenate(
            [S // RSC * i + RWC * c + np.arange(RWC)
             for i in range(RSC)]
        )
        # mlpT host layout [MT, 128, D]: H[m, p, t*128+f] = mlpT_all[t*128+p,
        # m*128+f] -> reshape/transpose
        mslab = mlpT_all[:, ml_sl].reshape(KT, 128, MT, 128)
        mslab = np.ascontiguousarray(
            mslab.transpose(2, 1, 0, 3).reshape(MT, 128, D))
        m = {
            "xTb": xTb,
            "xres": np.ascontiguousarray(x[sres]),
            "tembT": tembT,
            "outb_row": np.ascontiguousarray(
                np.asarray(out_b).astype(f32).reshape(1, D)),
            "normT": _bf16(normT_all[:, nm_sl]),
            "normb": np.ascontiguousarray(
                np.asarray(norm_b).astype(f32)[nm_sl].reshape(NJ, 128).T),
            "qwT": _bf16(qwT_all[:, qk_cols]),
            "kwT": _bf16(kwT_all[:, qk_cols]),
            "vwT": _bf16(vwT_all[:, v_cols]),
            "qb": np.ascontiguousarray(
                np.asarray(qb).astype(f32)[qk_cols].reshape(HPC, 128).T),
            "kb": np.ascontiguousarray(
                np.asarray(kb).astype(f32)[qk_cols].reshape(HPC, 128).T),
            "vb": np.ascontiguousarray(
                np.asarray(vb).astype(f32)[v_cols].reshape(HPC, 128).T),
            "qrw": np.ascontiguousarray(
                np.asarray(q_rms_w).astype(f32)[perm].reshape(128, 1)),
            "krw": np.ascontiguousarray(
                np.asarray(k_rms_w).astype(f32)[perm].reshape(128, 1)),
            "mlpT": _bf16(mslab),
            "mlpb": np.ascontiguousarray(
                np.asarray(mlp_b).astype(f32)[ml_sl].reshape(MT, 128).T),
            "outT": _bf16(outT_all[out_rows, :]),
            "cosP": cosP_b,
            "sinP": sinP_b,
        }
        in_maps.append(m)
    return in_maps


def run(inputs, debug=False, trace=False):
    nc = _build(debug=debug)
    in_maps = _prep_inputs(**inputs)
    res = run_bass_kernel_spmd(nc, in_maps, list(range(NC)), trace=trace)
    out = np.empty((S, D), np.float32)
    for c in range(NC):
        ys = res.results[c]["y"]
        for i in range(RSC):
            r0 = S // RSC * i + RWC * c
            out[r0 : r0 + RWC] = ys[RWC * i : RWC * (i + 1)]
    return out.reshape(B, S, D), res


def kernel(**inputs):
    out, _ = run(inputs)
    return out
